# revision 1
# baseline (speedup 1.0000x reference)
"""Trainium2 Bass kernel for nn_DAC_558345749225 (dynamic rotated grouped conv).

Reference (per sample b):
  pooled = mean_{H,W} x[b]                                  [C]
  angles = tanh(relu(pooled@W1^T+b1)@W2^T+b2) * pi/4        [G]
  rot[g] = bilinear-rotate(base_kernel[g], angles[g])       [Cg,Cg,3,3]
  feat   = grouped_conv3x3(x[b], rot, groups=G, pad=1)
  mod    = sigmoid(relu(pooled@M1^T+bm1)@M2^T+bm2)          [C]
  out    = feat * mod[:,None,None]

Sharding: data-parallel over batch — 2 samples per core on 8 cores.

Per-core pipeline (samples b=0,1; packs p=0,1 of 4 groups each):
  - x is zero-padded to 66x66 on the host and DMAd contiguously into SBUF
    tiles [128ch, 4356] declared float32r (raw fp32 bits; the PE rounds
    internally — verified bitwise-identical to pre-rounded inputs).
  - pooling: free-dim reduce over the padded row (border zeros don't change
    the sum); the 1/4096 scale is folded into the MLP weights host-side.
    Sample 0 reduces on DVE, sample 1 on GpSimd so they run concurrently.
  - per-sample: tiny MLPs on PE + ACT (Relu/Tanh/Sigmoid, cos via
    Sin(x+pi/2)); the 9x9 rotation map R[ij,mn](theta) built with ~22
    elementwise DVE ops on an [8=(g), 81=(mn,ij)] layout.
  - per (b,pack): R scattered into block-diag A [36,36]; one fp32 matmul
      out1[(g,ij),(ci,co)] = A^T @ Bmat   (Bmat = host-rearranged base_kernel)
    out1 rounded to float32r by the ACT copy out of PSUM, then 36 small sync
    DMAs scatter it into block-diag conv weights lhsT [128=(g,ci), 9*128].
  - conv: per (b,pack), 8 chunks of 8 output rows; 9 shift matmuls (float32r,
    1 PE cycle/row, 512 moving rows) accumulate into one PSUM bank; epilogue
    on ACT multiplies by the sigmoid gate (per-partition scale) and the
    result is DMAd to DRAM.
"""
import math
import numpy as np

N_CORES = 8
B, C, H, W = 16, 256, 64, 64
G, Cg = 8, 32
HID = 64
Bc = B // N_CORES          # samples per core = 2
NU = Bc * 2                # (b, pack) units per core = 4
HP, WP = H + 2, W + 2      # padded 66 x 66
NPIX = HP * WP             # 4356

_CACHE = {}


def _host_consts(base_kernel, ap_w1, ap_b1, ap_w2, ap_b2,
                 meta_w1, meta_b1, meta_w2, meta_b2):
    f32 = np.float32
    # Bmat [2, 36, 1024]: [p][g4*9+mn][ci*32+co] = base[4p+g4, co, ci, m, n]
    bk = np.asarray(base_kernel, f32)                      # [G, co, ci, 3, 3]
    bm = np.transpose(bk, (0, 3, 4, 2, 1))                 # [G, m, n, ci, co]
    bmat = np.ascontiguousarray(bm.reshape(2, 36, Cg * Cg))

    # R-build constants over free layout f = mn*9 + ij (mn-major)
    f = np.arange(81)
    mn, ij = f // 9, f % 9
    i, j = ij // 3, ij % 3
    m, n = mn // 3, mn % 3
    xx = (j - 1).astype(f32)
    yy = (i - 1).astype(f32)
    nv = n.astype(f32)
    mv = m.astype(f32)
    # packed layout: P=[xx|yy], Q=[yy|-xx], NM=[nv|mv] (x-half drives cos/U,
    # y-half drives V; y_rot = yy*c - xx*s)
    blocks = [
        np.concatenate([xx, yy]),                          # P   [162]
        np.concatenate([yy, -xx]),                         # Q   [162]
        np.concatenate([nv, mv]),                          # NM  [162]
    ]
    consts = np.tile(np.concatenate(blocks)[None, :], (16, 1)).astype(f32)

    scale = f32(1.0 / (H * W))
    w1T = np.asarray(ap_w1, f32).T * scale                 # [256, 64]
    m1T = np.asarray(meta_w1, f32).T * scale
    mlp1 = np.ascontiguousarray(np.concatenate(
        [w1T[:128], w1T[128:], m1T[:128], m1T[128:]], axis=1), f32)  # [128,256]
    w2T = np.asarray(ap_w2, f32).T                         # [64, 8]
    m2T = np.asarray(meta_w2, f32).T                       # [64, 256]
    mlp2 = np.ascontiguousarray(np.concatenate([w2T, m2T], axis=1), f32)
    bias64 = np.ascontiguousarray(
        np.stack([np.asarray(ap_b1, f32), np.asarray(meta_b1, f32)], axis=1))
    b2v = np.asarray(ap_b2, f32).reshape(8, 1).copy()
    mb2v = np.ascontiguousarray(np.asarray(meta_b2, f32).reshape(2, 128).T)
    return dict(bmat=bmat, consts=consts, mlp1=mlp1, mlp2=mlp2,
                bias64=bias64, b2v=b2v, mb2v=mb2v)


def _build_nc():
    import concourse.bass as bass
    import concourse.tile as tile
    from concourse import bacc, mybir
    dt = mybir.dt
    AF = mybir.ActivationFunctionType
    OP = mybir.AluOpType

    nc = bacc.Bacc("TRN2", target_bir_lowering=False, debug=False,
                   enable_asserts=False, num_devices=N_CORES)

    xs = nc.dram_tensor("xs", [Bc, C, HP, WP], dt.float32r, kind="ExternalInput").ap()
    bmat_d = nc.dram_tensor("bmat", [2, 36, 1024], dt.float32, kind="ExternalInput").ap()
    consts_d = nc.dram_tensor("consts", [16, 486], dt.float32, kind="ExternalInput").ap()
    mlp1_d = nc.dram_tensor("mlp1", [128, 256], dt.float32, kind="ExternalInput").ap()
    mlp2_d = nc.dram_tensor("mlp2", [64, 264], dt.float32, kind="ExternalInput").ap()
    bias64_d = nc.dram_tensor("bias64", [64, 2], dt.float32, kind="ExternalInput").ap()
    b2v_d = nc.dram_tensor("b2v", [8, 1], dt.float32, kind="ExternalInput").ap()
    mb2v_d = nc.dram_tensor("mb2v", [128, 2], dt.float32, kind="ExternalInput").ap()
    y = nc.dram_tensor("y", [Bc, C, H, W], dt.float32, kind="ExternalOutput").ap()

    xs_flat = xs.rearrange("b c h w -> (b c) (h w)")
    y_flat = y.rearrange("b c h w -> (b c) (h w)")

    with tile.TileContext(nc) as tc:
        from contextlib import ExitStack
        ctx = ExitStack()
        cpool = ctx.enter_context(tc.tile_pool(name="cpool", bufs=1))
        xpool = ctx.enter_context(tc.tile_pool(name="xpool", bufs=1))
        wpool = ctx.enter_context(tc.tile_pool(name="wpool", bufs=1))
        o1pool = ctx.enter_context(tc.tile_pool(name="o1pool", bufs=2))
        apool = ctx.enter_context(tc.tile_pool(name="apool", bufs=1))
        outpool = ctx.enter_context(tc.tile_pool(name="outpool", bufs=5))
        pconv = ctx.enter_context(tc.tile_pool(name="pconv", bufs=6, space="PSUM"))
        psmall = ctx.enter_context(tc.tile_pool(name="psmall", bufs=2, space="PSUM"))
        dscr = ctx.enter_context(tc.tile_pool(name="dscr", bufs=2, space="DRAM"))

        # ---------- constants ----------
        consts_t = cpool.tile([16, 486], dt.float32)
        nc.sync.dma_start(consts_t[:], consts_d[:])
        PC, QC, NM = (consts_t[:, 162 * k:162 * (k + 1)] for k in range(3))
        mlp1_t = cpool.tile([128, 256], dt.float32)
        nc.sync.dma_start(mlp1_t[:], mlp1_d[:])
        mlp2_t = cpool.tile([64, 264], dt.float32)
        nc.sync.dma_start(mlp2_t[:], mlp2_d[:])
        bias64_t = cpool.tile([64, 2], dt.float32)
        nc.sync.dma_start(bias64_t[:], bias64_d[:])
        b2v_t = cpool.tile([8, 1], dt.float32)
        nc.sync.dma_start(b2v_t[:], b2v_d[:])
        mb2v_t = cpool.tile([128, 2], dt.float32)
        nc.sync.dma_start(mb2v_t[:], mb2v_d[:])
        bsb = cpool.tile([36, 2048], dt.float32)
        nc.sync.dma_start(bsb[:, 0:1024], bmat_d[0])
        nc.sync.dma_start(bsb[:, 1024:2048], bmat_d[1])
        halfpi = cpool.tile([8, 1], dt.float32)
        nc.gpsimd.memset(halfpi[:], math.pi / 2)
        # pre-warm the ACT function tables (table loads cost ~1.3us each and
        # would otherwise land on the angle-chain critical path)
        warm = cpool.tile([1, 1], dt.float32)
        for fn in (AF.Relu, AF.Tanh, AF.Sin, AF.Sigmoid):
            nc.scalar.activation(warm[:], halfpi[0:1, 0:1], fn)

        # ---------- x loads (contiguous, two halves per tile) ----------
        # sample 0 loads immediately on sync; sample-1 loads are gated on
        # sample-0 pooling (dummy write) and issued from gpsimd so they don't
        # steal HBM bandwidth from sample 0 or head-of-line-block the sync
        # engine.
        HH = NPIX // 2                 # 2178
        QQ = NPIX // 4                 # 1089
        x_tiles = []
        for u in range(NU):
            b, p = divmod(u, 2)
            xt = xpool.tile([128, NPIX], dt.float32r, name=f"xt{u}")
            x_tiles.append(xt)
            if b == 0:
                src = xs_flat[128 * p:128 * (p + 1), :]
                for q in range(4):
                    nc.sync.dma_start(xt[:, QQ * q:QQ * (q + 1)],
                                      src[:, QQ * q:QQ * (q + 1)])

        # pre-allocate + zero the weight and A tiles up front (gpsimd is idle
        # here; the zeros are off-diagonal blocks that are never rewritten)
        lts = {}
        a_ts = {}
        for u in range(NU):
            lt = wpool.tile([128, 9 * 128], dt.float32r, name=f"lt{u}")
            nc.gpsimd.memset(lt[:].bitcast(dt.float32), 0.0)
            lts[u] = lt
            a_t = apool.tile([36, 36], dt.float32, name=f"a{u}")
            nc.gpsimd.memset(a_t[:], 0.0)
            a_ts[u] = a_t

        gate_scr = cpool.tile([1, 1], dt.float32)

        def emit_late_xload(u):
            b, p = divmod(u, 2)
            xt = x_tiles[u]
            # gate: dummy READ of the tile (WAR: the DMA write below must wait
            # for it) that also reads sample-0 pooling (RAW: waits for it)
            nc.gpsimd.tensor_scalar(gate_scr[:], xt[0:1, 0:1].bitcast(dt.float32),
                                    pooled[0:1, 2 * (u - 2):2 * (u - 2) + 1],
                                    None, op0=OP.mult)
            src = xs_flat[b * C + 128 * p:b * C + 128 * (p + 1), :]
            nc.gpsimd.dma_start(xt[:, 0:HH], src[:, 0:HH])
            nc.gpsimd.dma_start(xt[:, HH:NPIX], src[:, HH:NPIX])

        # ---------- pooling (emitted per-sample inside the loop below) ----------
        # pooled col layout: col = 2*pack + b
        pooled = cpool.tile([128, NU], dt.float32)
        pp = cpool.tile([128, 2 * NU], dt.float32)

        pq = cpool.tile([128, 4 * NU], dt.float32)

        def emit_pooling(b):
            for p in range(2):
                u = 2 * b + p
                xf = x_tiles[u][:].bitcast(dt.float32)
                if b == 0:
                    for q in range(4):
                        nc.vector.reduce_sum(pq[:, 4 * u + q:4 * u + q + 1],
                                             xf[:, QQ * q:QQ * (q + 1)],
                                             axis=mybir.AxisListType.X)
                    nc.vector.tensor_tensor(pp[:, 2 * u:2 * u + 1],
                                            pq[:, 4 * u:4 * u + 1],
                                            pq[:, 4 * u + 1:4 * u + 2], op=OP.add)
                    nc.vector.tensor_tensor(pp[:, 2 * u + 1:2 * u + 2],
                                            pq[:, 4 * u + 2:4 * u + 3],
                                            pq[:, 4 * u + 3:4 * u + 4], op=OP.add)
                else:
                    nc.vector.reduce_sum(pp[:, 2 * u:2 * u + 1], xf[:, 0:HH],
                                         axis=mybir.AxisListType.X)
                    nc.vector.reduce_sum(pp[:, 2 * u + 1:2 * u + 2], xf[:, HH:NPIX],
                                         axis=mybir.AxisListType.X)
                nc.vector.tensor_tensor(pooled[:, 2 * p + b:2 * p + b + 1],
                                        pp[:, 2 * u:2 * u + 1],
                                        pp[:, 2 * u + 1:2 * u + 2], op=OP.add)

        # ---------- per-sample prep + conv, staged so the PE stream never
        # waits on the *other* sample's prep ----------
        # PE stream: mlp0, rot0, gate0, conv-u0[0..5], mlp1, conv-u0[6..7],
        #            rot1, gate1, conv-u1, conv-u2, conv-u3
        mod_sb = cpool.tile([128, NU], dt.float32)   # col = 2*pack + b

        def vt(nm):
            return cpool.tile([8, 162], dt.float32, name=nm)

        TT = nc.vector.tensor_tensor
        TS = nc.vector.tensor_scalar
        STT = nc.vector.scalar_tensor_tensor

        def emit_angle_mlp(b):
            h_ps = psmall.tile([64, 1], dt.float32, tag="mlp", name=f"hps{b}")
            nc.tensor.matmul(h_ps[:], mlp1_t[:, 0:64], pooled[:, b:b + 1],
                             start=True, stop=False)
            nc.tensor.matmul(h_ps[:], mlp1_t[:, 64:128], pooled[:, 2 + b:3 + b],
                             start=False, stop=True)
            h_sb = cpool.tile([64, 1], dt.float32, name=f"hsb{b}")
            nc.scalar.activation(h_sb[:], h_ps[:], AF.Relu, bias=bias64_t[:, 0:1])
            ang_ps = psmall.tile([8, 1], dt.float32, tag="mlp", name=f"aps{b}")
            nc.tensor.matmul(ang_ps[:], mlp2_t[:, 0:8], h_sb[:], start=True, stop=True)
            ang_t = cpool.tile([8, 1], dt.float32, name=f"angt{b}")
            nc.scalar.activation(ang_t[:], ang_ps[:], AF.Tanh, bias=b2v_t[:])
            ang_sb = cpool.tile([8, 1], dt.float32, name=f"angs{b}")
            nc.vector.tensor_scalar_mul(ang_sb[:], ang_t[:], math.pi / 4)
            c_sb = cpool.tile([8, 1], dt.float32, name=f"csb{b}")
            nc.scalar.activation(c_sb[:], ang_sb[:], AF.Sin, bias=halfpi[:])
            s_sb = cpool.tile([8, 1], dt.float32, name=f"ssb{b}")
            nc.scalar.activation(s_sb[:], ang_sb[:], AF.Sin)
            return c_sb, s_sb

        def emit_r_build(b, c_sb, s_sb):
            t1, t2, vxy, av, ff, avp, u0t, u1t, du = (
                vt(f"r{k}_{b}") for k in range(9))
            r_b = cpool.tile([8, 81], dt.float32, name=f"rall_{b}")
            P8, Q8, NM8 = PC[0:8], QC[0:8], NM[0:8]
            nc.vector.tensor_scalar_mul(t1[:], P8, c_sb[:])
            nc.vector.tensor_scalar_mul(t2[:], Q8, s_sb[:])
            TT(vxy[:], t1[:], t2[:], op=OP.add)
            TS(av[:], vxy[:], 0.0, None, op0=OP.is_ge)
            STT(ff[:], vxy[:], 1.0, av[:], op0=OP.add, op1=OP.subtract)
            nc.vector.tensor_scalar_add(avp[:], av[:], 1.0)
            TT(u0t[:], NM8, av[:], op=OP.is_equal)
            TT(u1t[:], NM8, avp[:], op=OP.is_equal)
            TT(du[:], u1t[:], u0t[:], op=OP.subtract)
            TT(du[:], du[:], ff[:], op=OP.mult)
            TT(du[:], du[:], u0t[:], op=OP.add)
            TT(r_b[:], du[:, 0:81], du[:, 81:162], op=OP.mult)
            return r_b

        def emit_rotation(b, r_b):
            for p in range(2):
                u = 2 * b + p
                a_t = a_ts[u]
                for g4 in range(4):
                    r = 4 * p + g4
                    nc.sync.dma_start(
                        a_t[:][9 * g4:9 * (g4 + 1), 9 * g4:9 * (g4 + 1)],
                        r_b[:][r:r + 1].rearrange("q (mn ij) -> q mn ij", ij=9))
                o1_t = o1pool.tile([36, 1024], dt.float32r, tag="o1", name=f"o1{u}")
                for hh in range(2):
                    rot_ps = psmall.tile([36, 512], dt.float32, tag="mlp",
                                         name=f"rps{u}{hh}")
                    nc.tensor.matmul(
                        rot_ps[:], a_t[:],
                        bsb[:, 1024 * p + 512 * hh:1024 * p + 512 * (hh + 1)],
                        start=True, stop=True)
                    nc.scalar.copy(o1_t[:, 512 * hh:512 * (hh + 1)], rot_ps[:])
                wd = dscr.tile([36, 1024], dt.float32r, tag="wd", name=f"wd{u}")
                nc.sync.dma_start(wd[:], o1_t[:])
                wv = wd[:].rearrange("(g ij) (ci co) -> g ij ci co", ij=9, co=32)
                lt = lts[u]
                for g4 in range(4):
                    dst = lt[:][32 * g4:32 * (g4 + 1)].rearrange(
                        "q (ij co) -> q ij co", co=128)[:, :, 32 * g4:32 * (g4 + 1)]
                    nc.sync.dma_start(dst, wv[g4].transpose([1, 0, 2]))

        def emit_gate_mlp(b):
            m_ps = psmall.tile([64, 1], dt.float32, tag="mlp", name=f"mps{b}")
            nc.tensor.matmul(m_ps[:], mlp1_t[:, 128:192], pooled[:, b:b + 1],
                             start=True, stop=False)
            nc.tensor.matmul(m_ps[:], mlp1_t[:, 192:256], pooled[:, 2 + b:3 + b],
                             start=False, stop=True)
            m_sb = cpool.tile([64, 1], dt.float32, name=f"msb{b}")
            nc.scalar.activation(m_sb[:], m_ps[:], AF.Relu, bias=bias64_t[:, 1:2])
            for p in range(2):
                mod_ps = psmall.tile([128, 1], dt.float32, tag="mlp",
                                     name=f"modps{b}{p}")
                nc.tensor.matmul(mod_ps[:],
                                 mlp2_t[:, 8 + 128 * p:8 + 128 * (p + 1)],
                                 m_sb[:], start=True, stop=True)
                nc.scalar.activation(mod_sb[:, 2 * p + b:2 * p + b + 1], mod_ps[:],
                                     AF.Sigmoid, bias=mb2v_t[:, p:p + 1])

        NCH = 8

        def emit_conv_chunks(u, chunks):
            b, p = divmod(u, 2)
            x3 = x_tiles[u][:].rearrange("c (h w) -> c h w", w=WP)
            mod_col = mod_sb[:, 2 * p + b:2 * p + b + 1]
            lt = lts[u]
            for c8 in chunks:
                ps = pconv.tile([128, 512], dt.float32, tag="cps", name=f"cps{u}_{c8}")
                for sft in range(9):
                    ky, kx = divmod(sft, 3)
                    rhs = x3[:, c8 * 8 + ky:c8 * 8 + ky + 8, kx:kx + W]
                    nc.tensor.matmul(ps[:], lt[:, 128 * sft:128 * (sft + 1)],
                                     rhs, start=(sft == 0), stop=(sft == 8))
                ot = outpool.tile([128, 512], dt.float32, tag="ot", name=f"ot{u}_{c8}")
                nc.scalar.mul(ot[:], ps[:], mod_col)
                nc.scalar.dma_start(
                    y_flat[b * C + 128 * p:b * C + 128 * (p + 1),
                           512 * c8:512 * (c8 + 1)],
                    ot[:])

        # ---- sample 0 prep ----
        emit_pooling(0)
        emit_late_xload(2)
        emit_late_xload(3)
        c0, s0 = emit_angle_mlp(0)
        r0 = emit_r_build(0, c0, s0)
        emit_rotation(0, r0)
        emit_gate_mlp(0)
        # ---- conv unit 0 (first 6 chunks), with sample-1 prep interleaved ----
        emit_conv_chunks(0, range(0, 6))
        emit_pooling(1)
        c1, s1 = emit_angle_mlp(1)
        r1 = emit_r_build(1, c1, s1)
        emit_conv_chunks(0, range(6, 8))
        emit_rotation(1, r1)
        emit_gate_mlp(1)
        emit_conv_chunks(1, range(0, 8))
        emit_conv_chunks(2, range(0, 8))
        emit_conv_chunks(3, range(0, 8))
        ctx.close()

    nc.compile()
    return nc


def _get_nc():
    if "nc" not in _CACHE:
        _CACHE["nc"] = _build_nc()
    return _CACHE["nc"]


def _pad_x(x):
    xp = np.zeros((Bc, C, HP, WP), np.float32)
    xp[:, :, 1:H + 1, 1:W + 1] = x
    return xp


def run_on_device(inputs, trace=False, tmpdir=None):
    """Shard, run on 8 cores, gather. Returns (y_full, BassKernelResults)."""
    from concourse.bass_utils import run_bass_kernel_spmd
    x = np.ascontiguousarray(np.asarray(inputs["x"], np.float32))
    hc = _host_consts(
        inputs["base_kernel"], inputs["ap_w1"], inputs["ap_b1"],
        inputs["ap_w2"], inputs["ap_b2"], inputs["meta_w1"],
        inputs["meta_b1"], inputs["meta_w2"], inputs["meta_b2"])
    nc = _get_nc()
    xpad_full = np.zeros((B, C, HP, WP), np.float32)
    xpad_full[:, :, 1:H + 1, 1:W + 1] = x
    in_maps = []
    for c in range(N_CORES):
        im = {"xs": np.ascontiguousarray(xpad_full[Bc * c:Bc * (c + 1)])}
        im.update(hc)
        in_maps.append(im)
    kw = {}
    if trace:
        kw = dict(trace=True, tmpdir=tmpdir)
    res = run_bass_kernel_spmd(nc, in_maps, core_ids=list(range(N_CORES)), **kw)
    y = np.concatenate([res.results[c]["y"] for c in range(N_CORES)], axis=0)
    return y, res


def kernel(**inputs):
    y, _ = run_on_device(inputs)
    return y



# revision 8
# speedup vs baseline: 1.2480x; 1.2480x over previous
"""Trainium2 Bass kernel for nn_DAC_558345749225 (dynamic rotated grouped conv).

Reference (per sample b):
  pooled = mean_{H,W} x[b]                                  [C]
  angles = tanh(relu(pooled@W1^T+b1)@W2^T+b2) * pi/4        [G]
  rot[g] = bilinear-rotate(base_kernel[g], angles[g])       [Cg,Cg,3,3]
  feat   = grouped_conv3x3(x[b], rot, groups=G, pad=1)
  mod    = sigmoid(relu(pooled@M1^T+bm1)@M2^T+bm2)          [C]
  out    = feat * mod[:,None,None]

Sharding: data-parallel over batch - 2 samples per core on 8 cores.

v2 design (vs the 147us fp32 baseline):
  - x is zero-padded to 66x66 AND cast to bf16 on the host: halves the
    input DMA (4.45MB -> 2.2MB per sample per core), doubles DVE pooling
    throughput, and enables Fast Weight Load on the conv matmuls. The
    conv accumulates in fp32 PSUM; rel err ~5e-3 vs the 2e-2 gate.
  - the per-sample rotation matrix A is built entirely on-chip with DVE
    ops on a [72=(g,mn), *] layout (the tiny MLP's output layer is
    replicated 9x host-side so angles land on 72 partitions directly);
    no small scatter DMAs on the angle path.
  - one [72,72] x [72,1024] float32r rotation matmul per sample covers
    both 4-group packs; the (ij <-> ci) weight transpose still requires
    a DRAM roundtrip (SBUF APs cannot put the partition dim mid-AP),
    but it is one contiguous write + 8 group-scatter reads split across
    the two HW DGE queues, and only pack 0 of sample 0 is ever on the
    critical path.
  - schedule: x loads issue first on the sync queue; consts load on the
    scalar queue; sample-1 x loads issue from gpsimd (SWDGE), gated on
    sample-0 pooling so they never steal HBM bandwidth from the head.
    Sample-1 prep matmuls are emitted between conv chunks of unit 0/1 so
    the in-order PE queue never waits on data that is not ready.
  - conv: per (sample, pack) unit, 8 chunks of 8 output rows; 9 shift
    matmuls (bf16, 1 col/cycle, N=512) accumulate into one PSUM bank;
    ACT epilogue multiplies by the sigmoid gate and output DMAs
    alternate between the sync and scalar queues.
"""
import math
import numpy as np

N_CORES = 8
B, C, H, W = 16, 256, 64, 64
G, Cg = 8, 32
HID = 64
Bc = B // N_CORES          # samples per core = 2
HP, WP = H + 2, W + 2      # padded 66 x 66
NPIX = HP * WP             # 4356
HH = NPIX // 2             # 2178

_CACHE = {}


def _host_consts(base_kernel, ap_w1, ap_b1, ap_w2, ap_b2,
                 meta_w1, meta_b1, meta_w2, meta_b2):
    f32 = np.float32
    # bmat8 [72, 1024]: [g*9+mn][ci*32+co] = base[g, co, ci, m, n]
    bk = np.asarray(base_kernel, f32)                      # [G, co, ci, 3, 3]
    bm = np.transpose(bk, (0, 3, 4, 2, 1))                 # [G, m, n, ci, co]
    bmat8 = np.ascontiguousarray(bm.reshape(72, Cg * Cg))

    # R-build constants on [72=(g,mn), 18=(half, ij)] layout
    p = np.arange(72)
    mn = p % 9
    m, n = mn // 9 * 0 + mn // 3, mn % 3
    ij = np.arange(9)
    i, j = ij // 3, ij % 3
    xx = (j - 1).astype(f32)                               # [9]
    yy = (i - 1).astype(f32)
    ones9 = np.ones(9, f32)
    PC = np.concatenate([np.tile(xx, (72, 1)), np.tile(yy, (72, 1))], axis=1)
    QC = np.concatenate([np.tile(yy, (72, 1)), np.tile(-xx, (72, 1))], axis=1)
    NM = np.concatenate([n.astype(f32)[:, None] * ones9[None, :],
                         m.astype(f32)[:, None] * ones9[None, :]], axis=1)
    # block-diag mask: mask72[p, 9g+ij] = 1 if p//9 == g else 0
    mask72 = np.zeros((72, 72), f32)
    for g in range(8):
        mask72[9 * g:9 * (g + 1), 9 * g:9 * (g + 1)] = 1.0
    consts72 = np.ascontiguousarray(
        np.concatenate([PC, QC, NM, mask72], axis=1), f32)  # [72, 126]

    scale = f32(1.0 / (H * W))
    w1T = np.asarray(ap_w1, f32).T * scale                 # [256, 64]
    m1T = np.asarray(meta_w1, f32).T * scale
    mlp1 = np.ascontiguousarray(np.concatenate(
        [w1T[:128], w1T[128:], m1T[:128], m1T[128:]], axis=1), f32)  # [128,256]
    w2T = np.asarray(ap_w2, f32).T                         # [64, 8]
    w2T_rep = np.repeat(w2T, 9, axis=1)                    # [64, 72]
    m2T = np.asarray(meta_w2, f32).T                       # [64, 256]
    mlp2 = np.ascontiguousarray(np.concatenate([w2T_rep, m2T], axis=1), f32)
    bias64 = np.ascontiguousarray(
        np.stack([np.asarray(ap_b1, f32), np.asarray(meta_b1, f32)], axis=1))
    b2rep = np.ascontiguousarray(
        np.repeat(np.asarray(ap_b2, f32), 9).reshape(72, 1))
    mb2v = np.ascontiguousarray(np.asarray(meta_b2, f32).reshape(2, 128).T)
    return dict(bmat8=bmat8, consts72=consts72, mlp1=mlp1, mlp2=mlp2,
                bias64=bias64, b2rep=b2rep, mb2v=mb2v)


def _build_nc():
    import concourse.bass as bass
    import concourse.tile as tile
    from concourse import bacc, mybir
    dt = mybir.dt
    AF = mybir.ActivationFunctionType
    OP = mybir.AluOpType

    nc = bacc.Bacc("TRN2", target_bir_lowering=False, debug=False,
                   enable_asserts=False, num_devices=N_CORES)

    xs = nc.dram_tensor("xs", [Bc, C, HP, WP], dt.bfloat16, kind="ExternalInput").ap()
    bmat8_d = nc.dram_tensor("bmat8", [72, 1024], dt.float32r, kind="ExternalInput").ap()
    consts_d = nc.dram_tensor("consts72", [72, 126], dt.float32, kind="ExternalInput").ap()
    mlp1_d = nc.dram_tensor("mlp1", [128, 256], dt.float32, kind="ExternalInput").ap()
    mlp2_d = nc.dram_tensor("mlp2", [64, 328], dt.float32, kind="ExternalInput").ap()
    bias64_d = nc.dram_tensor("bias64", [64, 2], dt.float32, kind="ExternalInput").ap()
    b2rep_d = nc.dram_tensor("b2rep", [72, 1], dt.float32, kind="ExternalInput").ap()
    mb2v_d = nc.dram_tensor("mb2v", [128, 2], dt.float32, kind="ExternalInput").ap()
    y = nc.dram_tensor("y", [Bc, C, H, W], dt.float32, kind="ExternalOutput").ap()

    xs_flat = xs.rearrange("b c h w -> (b c) (h w)")
    y_flat = y.rearrange("b c h w -> (b c) (h w)")

    with tile.TileContext(nc) as tc:
        from contextlib import ExitStack
        ctx = ExitStack()
        cpool = ctx.enter_context(tc.tile_pool(name="cpool", bufs=1))
        xpool = ctx.enter_context(tc.tile_pool(name="xpool", bufs=1))
        wpool = ctx.enter_context(tc.tile_pool(name="wpool", bufs=1))
        opool = ctx.enter_context(tc.tile_pool(name="opool", bufs=2))
        outpool = ctx.enter_context(tc.tile_pool(name="outpool", bufs=5))
        pconv = ctx.enter_context(tc.tile_pool(name="pconv", bufs=6, space="PSUM"))
        psmall = ctx.enter_context(tc.tile_pool(name="psmall", bufs=2, space="PSUM"))
        dscr = ctx.enter_context(tc.tile_pool(name="dscr", bufs=2, space="DRAM"))

        # ---------- x sample-0 loads first thing on the sync HW queue ----------
        x_tiles = []
        for u in range(4):
            xt = xpool.tile([128, NPIX], dt.bfloat16, name=f"xt{u}")
            x_tiles.append(xt)
        for t in range(2):       # sample 0, channel halves
            src = xs_flat[128 * t:128 * (t + 1), :]
            for h in range(2):
                nc.sync.dma_start(x_tiles[t][:, HH * h:HH * (h + 1)],
                                  src[:, HH * h:HH * (h + 1)])

        # ---------- constants on the scalar HW queue ----------
        bmat8_t = cpool.tile([72, 1024], dt.float32r)
        nc.scalar.dma_start(bmat8_t[:], bmat8_d[:])
        consts_t = cpool.tile([72, 126], dt.float32)
        nc.scalar.dma_start(consts_t[:], consts_d[:])
        PC, QC, NM = (consts_t[:, 18 * k:18 * (k + 1)] for k in range(3))
        MASK = consts_t[:, 54:126]
        mlp1_t = cpool.tile([128, 256], dt.float32)
        nc.scalar.dma_start(mlp1_t[:], mlp1_d[:])
        mlp2_t = cpool.tile([64, 328], dt.float32)
        nc.scalar.dma_start(mlp2_t[:], mlp2_d[:])
        bias64_t = cpool.tile([64, 2], dt.float32)
        nc.scalar.dma_start(bias64_t[:], bias64_d[:])
        b2rep_t = cpool.tile([72, 1], dt.float32)
        nc.scalar.dma_start(b2rep_t[:], b2rep_d[:])
        mb2v_t = cpool.tile([128, 2], dt.float32)
        nc.scalar.dma_start(mb2v_t[:], mb2v_d[:])

        # ---------- memsets on gpsimd ----------
        halfpi = cpool.tile([72, 1], dt.float32)
        nc.gpsimd.memset(halfpi[:], math.pi / 2)
        lts = {}
        for u in range(4):
            lt = wpool.tile([128, 9 * 128], dt.bfloat16, name=f"lt{u}")
            nc.gpsimd.memset(lt[:], 0.0)
            lts[u] = lt
        a8s = {}
        for s in range(2):
            # fully written by the 8 masked mults in emit_r_build; no memset
            a8s[s] = cpool.tile([72, 72], dt.float32r, name=f"a8_{s}")
        gate_scr = cpool.tile([1, 1], dt.float32)
        nc.gpsimd.memset(gate_scr[:], 0.0)

        # pre-warm ACT function tables (~1.3us each; off the critical path here)
        warm = cpool.tile([1, 1], dt.float32)
        for fn in (AF.Relu, AF.Tanh, AF.Sin, AF.Sigmoid):
            nc.scalar.activation(warm[:], gate_scr[0:1, 0:1], fn)

        # ---------- pooling ----------
        # pooled col layout: col = 2*s + t (sample, channel-half)
        pooled = cpool.tile([128, 4], dt.float32)
        pp = cpool.tile([128, 8], dt.float32)

        def emit_pooling(s):
            for t in range(2):
                u = 2 * s + t
                xf = x_tiles[u][:]
                for h in range(2):
                    nc.vector.reduce_sum(pp[:, 2 * u + h:2 * u + h + 1],
                                         xf[:, HH * h:HH * (h + 1)],
                                         axis=mybir.AxisListType.X)
                nc.vector.tensor_tensor(pooled[:, u:u + 1],
                                        pp[:, 2 * u:2 * u + 1],
                                        pp[:, 2 * u + 1:2 * u + 2], op=OP.add)

        emit_pooling(0)

        # ---------- gated sample-1 x loads from gpsimd (SWDGE) ----------
        nc.gpsimd.tensor_scalar(gate_scr[:], gate_scr[:], pooled[0:1, 0:1],
                                None, op0=OP.mult)
        for t in range(2):
            src = xs_flat[C + 128 * t:C + 128 * (t + 1), :]
            for h in range(2):
                nc.gpsimd.dma_start(x_tiles[2 + t][:, HH * h:HH * (h + 1)],
                                    src[:, HH * h:HH * (h + 1)])

        # ---------- per-sample prep ----------
        TT = nc.vector.tensor_tensor
        TS = nc.vector.tensor_scalar
        STT = nc.vector.scalar_tensor_tensor
        mod_sb = cpool.tile([128, 4], dt.float32)   # col = 2*p + s

        def emit_angle_mlp(s):
            h_ps = psmall.tile([64, 1], dt.float32, tag="mlp", name=f"hps{s}")
            nc.tensor.matmul(h_ps[:], mlp1_t[:, 0:64], pooled[:, 2 * s:2 * s + 1],
                             start=True, stop=False)
            nc.tensor.matmul(h_ps[:], mlp1_t[:, 64:128], pooled[:, 2 * s + 1:2 * s + 2],
                             start=False, stop=True)
            h_sb = cpool.tile([64, 1], dt.float32, name=f"hsb{s}")
            nc.scalar.activation(h_sb[:], h_ps[:], AF.Relu, bias=bias64_t[:, 0:1])
            ang_ps = psmall.tile([72, 1], dt.float32, tag="mlp", name=f"aps{s}")
            nc.tensor.matmul(ang_ps[:], mlp2_t[:, 0:72], h_sb[:], start=True, stop=True)
            ang_t = cpool.tile([72, 1], dt.float32, name=f"angt{s}")
            nc.scalar.activation(ang_t[:], ang_ps[:], AF.Tanh, bias=b2rep_t[:])
            ang_sb = cpool.tile([72, 1], dt.float32, name=f"angs{s}")
            nc.vector.tensor_scalar_mul(ang_sb[:], ang_t[:], math.pi / 4)
            c_sb = cpool.tile([72, 1], dt.float32, name=f"csb{s}")
            nc.scalar.activation(c_sb[:], ang_sb[:], AF.Sin, bias=halfpi[:])
            s_sb = cpool.tile([72, 1], dt.float32, name=f"ssb{s}")
            nc.scalar.activation(s_sb[:], ang_sb[:], AF.Sin)
            return c_sb, s_sb

        def vt(nm):
            return cpool.tile([72, 18], dt.float32, name=nm)

        def emit_r_build(s, c_sb, s_sb):
            t1, t2, vxy, av, ff, avp, u0t, u1t, du = (
                vt(f"r{k}_{s}") for k in range(9))
            rc = cpool.tile([72, 9], dt.float32, name=f"rc{s}")
            nc.vector.tensor_scalar_mul(t1[:], PC, c_sb[:])
            nc.vector.tensor_scalar_mul(t2[:], QC, s_sb[:])
            TT(vxy[:], t1[:], t2[:], op=OP.add)
            TS(av[:], vxy[:], 0.0, None, op0=OP.is_ge)
            STT(ff[:], vxy[:], 1.0, av[:], op0=OP.add, op1=OP.subtract)
            nc.vector.tensor_scalar_add(avp[:], av[:], 1.0)
            TT(u0t[:], NM, av[:], op=OP.is_equal)
            TT(u1t[:], NM, avp[:], op=OP.is_equal)
            TT(du[:], u1t[:], u0t[:], op=OP.subtract)
            TT(du[:], du[:], ff[:], op=OP.mult)
            TT(du[:], du[:], u0t[:], op=OP.add)
            TT(rc[:], du[:, 0:9], du[:, 9:18], op=OP.mult)
            # block-diagonal A8 via masked mults (full 72-partition ops; DVE
            # partition offsets must be 32-aligned so per-block slices are out).
            # Written as float32r directly so the PE may consume it raw.
            a8v = a8s[s][:]
            for g in range(8):
                TT(a8v[:, 9 * g:9 * (g + 1)], rc[:], MASK[:, 9 * g:9 * (g + 1)],
                   op=OP.mult)

        def emit_rotation(s):
            """rot matmul + DRAM-roundtrip scatter into conv lhsT layout."""
            o1 = opool.tile([72, 1024], dt.bfloat16, tag="o1", name=f"o1_{s}")
            for h in range(2):
                rot_ps = psmall.tile([72, 512], dt.float32, tag="mlp",
                                     name=f"rps{s}{h}")
                nc.tensor.matmul(rot_ps[:], a8s[s][:],
                                 bmat8_t[:, 512 * h:512 * (h + 1)],
                                 start=True, stop=True)
                nc.scalar.copy(o1[:, 512 * h:512 * (h + 1)], rot_ps[:])
            wd = dscr.tile([72, 1024], dt.bfloat16, tag="wd", name=f"wd{s}")
            nc.sync.dma_start(wd[:], o1[:])
            wv = wd[:].rearrange("(g ij) (ci co) -> g ij ci co", ij=9, co=32)
            # pack-0 groups first; alternate queues so two run concurrently
            for k, g in enumerate((0, 1, 2, 3, 4, 5, 6, 7)):
                p, g4 = divmod(g, 4)
                lt = lts[2 * s + p]
                dst = lt[:][32 * g4:32 * (g4 + 1)].rearrange(
                    "q (ij co) -> q ij co", co=128)[:, :, 32 * g4:32 * (g4 + 1)]
                eng = nc.sync if k % 2 == 0 else nc.scalar
                eng.dma_start(dst, wv[g].transpose([1, 0, 2]))

        def emit_gate_mlp(s):
            m_ps = psmall.tile([64, 1], dt.float32, tag="mlp", name=f"mps{s}")
            nc.tensor.matmul(m_ps[:], mlp1_t[:, 128:192], pooled[:, 2 * s:2 * s + 1],
                             start=True, stop=False)
            nc.tensor.matmul(m_ps[:], mlp1_t[:, 192:256],
                             pooled[:, 2 * s + 1:2 * s + 2],
                             start=False, stop=True)
            m_sb = cpool.tile([64, 1], dt.float32, name=f"msb{s}")
            nc.scalar.activation(m_sb[:], m_ps[:], AF.Relu, bias=bias64_t[:, 1:2])
            for p in range(2):
                mod_ps = psmall.tile([128, 1], dt.float32, tag="mlp",
                                     name=f"modps{s}{p}")
                nc.tensor.matmul(mod_ps[:],
                                 mlp2_t[:, 72 + 128 * p:72 + 128 * (p + 1)],
                                 m_sb[:], start=True, stop=True)
                nc.scalar.activation(mod_sb[:, 2 * p + s:2 * p + s + 1], mod_ps[:],
                                     AF.Sigmoid, bias=mb2v_t[:, p:p + 1])

        def emit_conv_chunks(u, chunks):
            s, p = divmod(u, 2)
            x3 = x_tiles[u][:].rearrange("c (h w) -> c h w", w=WP)
            mod_col = mod_sb[:, 2 * p + s:2 * p + s + 1]
            lt = lts[u]
            for c8 in chunks:
                ps = pconv.tile([128, 512], dt.float32, tag="cps", name=f"cps{u}_{c8}")
                for sft in range(9):
                    ky, kx = divmod(sft, 3)
                    rhs = x3[:, c8 * 8 + ky:c8 * 8 + ky + 8, kx:kx + W]
                    nc.tensor.matmul(ps[:], lt[:, 128 * sft:128 * (sft + 1)],
                                     rhs, start=(sft == 0), stop=(sft == 8))
                ot = outpool.tile([128, 512], dt.float32, tag="ot", name=f"ot{u}_{c8}")
                nc.scalar.mul(ot[:], ps[:], mod_col)
                eng = nc.sync if c8 % 2 == 0 else nc.scalar
                eng.dma_start(
                    y_flat[s * C + 128 * p:s * C + 128 * (p + 1),
                           512 * c8:512 * (c8 + 1)],
                    ot[:])

        # ---- sample 0 prep ----
        c0, s0 = emit_angle_mlp(0)
        emit_r_build(0, c0, s0)
        emit_rotation(0)
        emit_gate_mlp(0)
        # pooling for sample 1 on DVE (behind r-build 0 in the DVE queue)
        emit_pooling(1)
        # ---- conv unit 0, with sample-1 prep interleaved into the PE stream
        # late enough that its dependencies are always ready ----
        emit_conv_chunks(0, range(0, 5))
        c1, s1 = emit_angle_mlp(1)
        emit_conv_chunks(0, range(5, 7))
        emit_r_build(1, c1, s1)
        emit_rotation(1)
        emit_conv_chunks(0, range(7, 8))
        emit_gate_mlp(1)
        emit_conv_chunks(1, range(0, 8))
        emit_conv_chunks(2, range(0, 8))
        emit_conv_chunks(3, range(0, 8))
        ctx.close()

    nc.compile()
    return nc


def _get_nc():
    if "nc" not in _CACHE:
        _CACHE["nc"] = _build_nc()
    return _CACHE["nc"]


def run_on_device(inputs, trace=False, tmpdir=None):
    """Shard, run on 8 cores, gather. Returns (y_full, BassKernelResults)."""
    import ml_dtypes
    from concourse.bass_utils import run_bass_kernel_spmd
    x = np.ascontiguousarray(np.asarray(inputs["x"], np.float32))
    hc = _host_consts(
        inputs["base_kernel"], inputs["ap_w1"], inputs["ap_b1"],
        inputs["ap_w2"], inputs["ap_b2"], inputs["meta_w1"],
        inputs["meta_b1"], inputs["meta_w2"], inputs["meta_b2"])
    nc = _get_nc()
    xpad_full = np.zeros((B, C, HP, WP), ml_dtypes.bfloat16)
    xpad_full[:, :, 1:H + 1, 1:W + 1] = x
    in_maps = []
    for c in range(N_CORES):
        im = {"xs": np.ascontiguousarray(xpad_full[Bc * c:Bc * (c + 1)])}
        im.update(hc)
        in_maps.append(im)
    kw = {}
    if trace:
        kw = dict(trace=True, tmpdir=tmpdir)
    res = run_bass_kernel_spmd(nc, in_maps, core_ids=list(range(N_CORES)), **kw)
    y = np.concatenate([res.results[c]["y"] for c in range(N_CORES)], axis=0)
    return y, res


def kernel(**inputs):
    y, _ = run_on_device(inputs)
    return y


# revision 13
# speedup vs baseline: 1.3807x; 1.1063x over previous
"""Trainium2 Bass kernel for nn_DAC_558345749225 (dynamic rotated grouped conv).

Reference (per sample b):
  pooled = mean_{H,W} x[b]                                  [C]
  angles = tanh(relu(pooled@W1^T+b1)@W2^T+b2) * pi/4        [G]
  rot[g] = bilinear-rotate(base_kernel[g], angles[g])       [Cg,Cg,3,3]
  feat   = grouped_conv3x3(x[b], rot, groups=G, pad=1)
  mod    = sigmoid(relu(pooled@M1^T+bm1)@M2^T+bm2)          [C]
  out    = feat * mod[:,None,None]

Sharding: data-parallel over batch - 2 samples per core on 8 cores.

v3 design notes (measured-trace driven):
  - x is zero-padded to 66x66 and cast to bf16 on the host (half the DMA
    bytes, fp32 PSUM accumulation; rel err ~2.4e-3 vs the 2e-2 gate).
  - every dma_start blocks its issuing engine 0.6-1.5us and there are
    only 8 DMA-completion semaphore lanes, so DMA count is minimized:
    one packed small-consts DMA, x0 as 4+4 quarter DMAs split across the
    two HW DGE queues (sync=tile0, scalar=tile1), conv-weight zeros as
    one DMA from a host zeros tensor instead of 4 gpsimd memsets.
  - pooling overlaps the x0 DMAs: DVE reduces tile-0 quarters as they
    land while gpsimd reduces tile-1; sample-1 pooling runs entirely on
    gpsimd so it can never be interleaved into the DVE queue ahead of
    the rotation-matrix build (the tile scheduler reorders queues).
  - ACT function-table thrash eliminated: Relu runs on DVE
    (add-bias/max-0 tensor_scalar) and Sigmoid becomes
    0.5*tanh(z/2)+0.5 with the halving folded into the host-side
    weights, so ACT only ever needs {Copy, Tanh, Sin} - warmed once.
  - the tiny MLP's output layer is replicated 9x host-side so angles
    land on 72=(g,mn) partitions; the 9x9 rotation matrix entries are
    built with ~18 DVE ops and masked into a block-diagonal [72,72]
    float32r A8 (no scatter DMAs on the angle path).
  - per-pack rotation matmuls A8[:,36p:36p+36]^T @ bmat8 -> [36,1024];
    the (ij<->ci) weight transpose needs a DRAM roundtrip (SBUF APs
    cannot put the partition dim mid-AP): per pack one contiguous write
    + 4 group-scatter reads split across both HW queues.  Only sample-0
    pack-0 is on the critical path.
  - sample-1 x loads issue from gpsimd (SWDGE) gated on sample-0
    pooling via a WAR read of the destination tiles (scheduler-proof).
  - HAM warm-up: a batch of dummy bf16 matmuls keeps the PE busy during
    the x0 load so the conv starts at the full 2.4 GHz clock.
  - conv: per (sample,pack) unit, 8 chunks of 8 output rows; 9 shift
    matmuls (bf16 lhsT enables fast-weight-load, N=512, 1 col/cycle)
    accumulate into one PSUM bank; ACT epilogue applies the sigmoid
    gate; output DMAs alternate between the sync and scalar queues.
"""
import math
import numpy as np

N_CORES = 8
B, C, H, W = 16, 256, 64, 64
G, Cg = 8, 32
HID = 64
Bc = B // N_CORES          # samples per core = 2
HP, WP = H + 2, W + 2      # padded 66 x 66
NPIX = HP * WP             # 4356
HH = NPIX // 2             # 2178
QQ = NPIX // 4             # 1089

NCOL = 715                 # packed small-consts columns
N_WARM = 16                # HAM warm-up dummy matmuls

_CACHE = {}


def _host_consts(base_kernel, ap_w1, ap_b1, ap_w2, ap_b2,
                 meta_w1, meta_b1, meta_w2, meta_b2):
    import ml_dtypes
    f32 = np.float32
    # bmat8 [72, 1024]: [g*9+mn][ci*32+co] = base[g, co, ci, m, n]
    bk = np.asarray(base_kernel, f32)                      # [G, co, ci, 3, 3]
    bm = np.transpose(bk, (0, 3, 4, 2, 1))                 # [G, m, n, ci, co]
    bmat8 = np.ascontiguousarray(bm.reshape(72, Cg * Cg))

    # R-build constants on [72=(g,mn), 18=(half, ij)] layout
    p = np.arange(72)
    mn = p % 9
    m, n = mn // 3, mn % 3
    ij = np.arange(9)
    i, j = ij // 3, ij % 3
    xx = (j - 1).astype(f32)                               # [9]
    yy = (i - 1).astype(f32)
    ones9 = np.ones(9, f32)
    PC = np.concatenate([np.tile(xx, (72, 1)), np.tile(yy, (72, 1))], axis=1)
    QC = np.concatenate([np.tile(yy, (72, 1)), np.tile(-xx, (72, 1))], axis=1)
    NM = np.concatenate([n.astype(f32)[:, None] * ones9[None, :],
                         m.astype(f32)[:, None] * ones9[None, :]], axis=1)
    mask72 = np.zeros((72, 72), f32)
    for g in range(8):
        mask72[9 * g:9 * (g + 1), 9 * g:9 * (g + 1)] = 1.0

    scale = f32(1.0 / (H * W))
    w1T = np.asarray(ap_w1, f32).T * scale                 # [256, 64]
    m1T = np.asarray(meta_w1, f32).T * scale
    mlp1 = np.concatenate(
        [w1T[:128], w1T[128:], m1T[:128], m1T[128:]], axis=1)      # [128,256]
    w2T_rep = np.repeat(np.asarray(ap_w2, f32).T, 9, axis=1)       # [64, 72]
    m2T_half = np.asarray(meta_w2, f32).T * f32(0.5)               # [64, 256]
    mlp2 = np.concatenate([w2T_rep, m2T_half], axis=1)             # [64, 328]
    bias64 = np.stack([np.asarray(ap_b1, f32), np.asarray(meta_b1, f32)],
                      axis=1)                                      # [64, 2]
    b2rep = np.repeat(np.asarray(ap_b2, f32), 9).reshape(72, 1)
    mb2v_half = np.asarray(meta_b2, f32).reshape(2, 128).T * f32(0.5)

    consts = np.zeros((128, NCOL), f32)
    consts[:, 0:256] = mlp1
    consts[0:64, 256:584] = mlp2
    consts[0:64, 584:586] = bias64
    consts[0:72, 586:587] = b2rep
    consts[:, 587:589] = mb2v_half
    consts[0:72, 589:607] = PC
    consts[0:72, 607:625] = QC
    consts[0:72, 625:643] = NM
    consts[0:72, 643:715] = mask72

    zeros_lt = np.zeros((128, 4608), ml_dtypes.bfloat16)
    return dict(bmat8=bmat8, consts=np.ascontiguousarray(consts),
                zeros_lt=zeros_lt)


def _build_nc():
    import concourse.bass as bass
    import concourse.tile as tile
    from concourse import bacc, mybir
    dt = mybir.dt
    AF = mybir.ActivationFunctionType
    OP = mybir.AluOpType

    nc = bacc.Bacc("TRN2", target_bir_lowering=False, debug=False,
                   enable_asserts=False, num_devices=N_CORES)

    xs = nc.dram_tensor("xs", [Bc, C, HP, WP], dt.bfloat16, kind="ExternalInput").ap()
    bmat8_d = nc.dram_tensor("bmat8", [72, 1024], dt.float32r, kind="ExternalInput").ap()
    consts_d = nc.dram_tensor("consts", [128, NCOL], dt.float32, kind="ExternalInput").ap()
    zeros_d = nc.dram_tensor("zeros_lt", [128, 4608], dt.bfloat16, kind="ExternalInput").ap()
    y = nc.dram_tensor("y", [Bc, C, H, W], dt.float32, kind="ExternalOutput").ap()

    xs_flat = xs.rearrange("b c h w -> (b c) (h w)")
    y_flat = y.rearrange("b c h w -> (b c) (h w)")

    with tile.TileContext(nc) as tc:
        from contextlib import ExitStack
        ctx = ExitStack()
        cpool = ctx.enter_context(tc.tile_pool(name="cpool", bufs=1))
        xpool = ctx.enter_context(tc.tile_pool(name="xpool", bufs=1))
        wpool = ctx.enter_context(tc.tile_pool(name="wpool", bufs=1))
        opool = ctx.enter_context(tc.tile_pool(name="opool", bufs=2))
        outpool = ctx.enter_context(tc.tile_pool(name="outpool", bufs=5))
        pconv = ctx.enter_context(tc.tile_pool(name="pconv", bufs=5, space="PSUM"))
        psmall = ctx.enter_context(tc.tile_pool(name="psmall", bufs=2, space="PSUM"))
        pdummy = ctx.enter_context(tc.tile_pool(name="pdummy", bufs=1, space="PSUM"))
        dscr = ctx.enter_context(tc.tile_pool(name="dscr", bufs=2, space="DRAM"))

        # ---------- x sample-0 loads: tile0 quarters on sync, tile1 on scalar
        x_tiles = [xpool.tile([128, NPIX], dt.bfloat16, name=f"xt{u}")
                   for u in range(4)]
        for t in range(2):
            src = xs_flat[128 * t:128 * (t + 1), :]
            eng = nc.sync if t == 0 else nc.scalar
            for q in range(4):
                eng.dma_start(x_tiles[t][:, QQ * q:QQ * (q + 1)],
                              src[:, QQ * q:QQ * (q + 1)])
        # bmat8 on sync behind the x0-tile0 quarters (needed ~rot time)
        bmat8_t = cpool.tile([72, 1024], dt.float32r)
        nc.sync.dma_start(bmat8_t[:], bmat8_d[:])

        # conv-weight zeros: one DMA into the mega lhsT tile (off-diagonal
        # blocks of the block-diag weights must be zero; scatter writes diag)
        ltbig = wpool.tile([128, 4608], dt.bfloat16)
        nc.scalar.dma_start(ltbig[:], zeros_d[:])

        # packed small consts via gpsimd (SWDGE) so the HW queues stay clear
        consts_t = cpool.tile([128, NCOL], dt.float32)
        nc.gpsimd.dma_start(consts_t[:], consts_d[:])
        MLP1 = consts_t[:, 0:256]
        MLP2 = consts_t[0:64, 256:584]
        BIAS64 = consts_t[0:64, 584:586]
        B2REP = consts_t[0:72, 586:587]
        MB2V = consts_t[:, 587:589]
        PC = consts_t[0:72, 589:607]
        QC = consts_t[0:72, 607:625]
        NM = consts_t[0:72, 625:643]
        MASK = consts_t[0:72, 643:715]

        # ---------- gpsimd memsets (warm tile first: dummies need it) ------
        warm_t = cpool.tile([128, 512], dt.bfloat16)
        nc.gpsimd.memset(warm_t[:], 0.0)
        halfpi = cpool.tile([72, 1], dt.float32)
        nc.gpsimd.memset(halfpi[:], math.pi / 2)
        scr = cpool.tile([1, 2], dt.float32)
        nc.gpsimd.memset(scr[:], 0.0)

        # ---------- ACT table warm: only {Tanh, Sin, Copy} are ever used ---
        warm_o = cpool.tile([1, 1], dt.float32)
        for fn in (AF.Tanh, AF.Sin):
            nc.scalar.activation(warm_o[:], scr[0:1, 0:1], fn)
        nc.scalar.copy(warm_o[:], scr[0:1, 0:1])

        # ---------- HAM warm-up dummies (PE busy during the x0 load) -------
        warm_ps = pdummy.tile([128, 512], dt.float32, name="warmps")
        for _ in range(N_WARM):
            nc.tensor.matmul(warm_ps[:], warm_t[:, 0:128], warm_t[:],
                             start=True, stop=True)

        # ---------- pooling ----------
        # pooled col = 2*s + t; pq: t0 q0..3 -> 0..3 (DVE), t1 -> 4..7
        # (gpsimd), t2 h0/h1 -> 8,9, t3 -> 10,11 (gpsimd)
        pooled = cpool.tile([128, 4], dt.float32)
        pq = cpool.tile([128, 12], dt.float32)
        pp = cpool.tile([128, 4], dt.float32)

        # sample-0 tile-0 on DVE (free-dim reduce is DVE-only)
        xf0 = x_tiles[0][:]
        for q in range(4):
            nc.vector.reduce_sum(pq[:, q:q + 1], xf0[:, QQ * q:QQ * (q + 1)],
                                 axis=mybir.AxisListType.X)
        nc.vector.tensor_tensor(pp[:, 0:1], pq[:, 0:1], pq[:, 1:2], op=OP.add)
        nc.vector.tensor_tensor(pp[:, 1:2], pq[:, 2:3], pq[:, 3:4], op=OP.add)
        nc.vector.tensor_tensor(pooled[:, 0:1], pp[:, 0:1], pp[:, 1:2], op=OP.add)
        # sample-0 tile-1 on ACT via Copy+accum_out (ACT is idle in the head);
        # the tiny pairwise adds run on gpsimd
        xf1 = x_tiles[1][:]
        junk = cpool.tile([128, QQ], dt.bfloat16)
        for q in range(4):
            nc.scalar.activation(junk[:], xf1[:, QQ * q:QQ * (q + 1)], AF.Copy,
                                 accum_out=pq[:, 4 + q:5 + q])
        nc.gpsimd.tensor_tensor(pp[:, 2:3], pq[:, 4:5], pq[:, 5:6], op=OP.add)
        nc.gpsimd.tensor_tensor(pp[:, 3:4], pq[:, 6:7], pq[:, 7:8], op=OP.add)
        nc.gpsimd.tensor_tensor(pooled[:, 1:2], pp[:, 2:3], pp[:, 3:4], op=OP.add)

        # ---------- gated sample-1 x loads from gpsimd (SWDGE) -------------
        # the gate op READS the destination tiles (WAR) so the scheduler
        # cannot hoist the DMAs above sample-0 pooling
        for t in range(2):
            xfv = x_tiles[2 + t][:].bitcast(dt.float32)
            nc.gpsimd.tensor_scalar(scr[:], xfv[0:1, HH // 2 - 1:HH // 2 + 1],
                                    pooled[0:1, 1:2], None, op0=OP.mult)
            src = xs_flat[C + 128 * t:C + 128 * (t + 1), :]
            for h in range(2):
                nc.gpsimd.dma_start(x_tiles[2 + t][:, HH * h:HH * (h + 1)],
                                    src[:, HH * h:HH * (h + 1)])
        def emit_pooling1():
            # sample-1 pooling on DVE; emitted after the r-build so the
            # greedy scheduler (which runs ready-work first) keeps the
            # r-build ahead of it: x1 lands after the r-build is ready.
            for t in range(2):
                xfv = x_tiles[2 + t][:]
                for h in range(2):
                    nc.vector.reduce_sum(pq[:, 8 + 2 * t + h:9 + 2 * t + h],
                                         xfv[:, HH * h:HH * (h + 1)],
                                         axis=mybir.AxisListType.X)
                nc.vector.tensor_tensor(pooled[:, 2 + t:3 + t],
                                        pq[:, 8 + 2 * t:9 + 2 * t],
                                        pq[:, 9 + 2 * t:10 + 2 * t], op=OP.add)

        # ---------- per-sample prep ----------
        TT = nc.vector.tensor_tensor
        TS = nc.vector.tensor_scalar
        STT = nc.vector.scalar_tensor_tensor
        mod_sb = cpool.tile([128, 4], dt.float32)   # col = 2*p + s
        a8s = {s: cpool.tile([72, 72], dt.float32r, name=f"a8_{s}")
               for s in range(2)}

        def emit_angle_mlp(s):
            h_ps = psmall.tile([64, 1], dt.float32, tag="mlp", name=f"hps{s}")
            nc.tensor.matmul(h_ps[:], MLP1[:, 0:64], pooled[:, 2 * s:2 * s + 1],
                             start=True, stop=False)
            nc.tensor.matmul(h_ps[:], MLP1[:, 64:128],
                             pooled[:, 2 * s + 1:2 * s + 2],
                             start=False, stop=True)
            h_sb = cpool.tile([64, 1], dt.float32, name=f"hsb{s}")
            TS(h_sb[:], h_ps[:], BIAS64[:, 0:1], 0.0, op0=OP.add, op1=OP.max)
            ang_ps = psmall.tile([72, 1], dt.float32, tag="mlp", name=f"aps{s}")
            nc.tensor.matmul(ang_ps[:], MLP2[:, 0:72], h_sb[:],
                             start=True, stop=True)
            ang_t = cpool.tile([72, 1], dt.float32, name=f"angt{s}")
            nc.scalar.activation(ang_t[:], ang_ps[:], AF.Tanh, bias=B2REP)
            c_sb = cpool.tile([72, 1], dt.float32, name=f"csb{s}")
            nc.scalar.activation(c_sb[:], ang_t[:], AF.Sin, bias=halfpi[:],
                                 scale=math.pi / 4)
            s_sb = cpool.tile([72, 1], dt.float32, name=f"ssb{s}")
            nc.scalar.activation(s_sb[:], ang_t[:], AF.Sin, scale=math.pi / 4)
            return c_sb, s_sb

        def vt(nm):
            return cpool.tile([72, 18], dt.float32, name=nm)

        def emit_r_build(s, c_sb, s_sb):
            t1, vxy, av, ff, u0t, u1t, du = (vt(f"r{k}_{s}") for k in range(7))
            rc = cpool.tile([72, 9], dt.float32, name=f"rc{s}")
            nc.vector.tensor_scalar_mul(t1[:], PC, c_sb[:])
            STT(vxy[:], QC, s_sb[:], t1[:], op0=OP.mult, op1=OP.add)
            TS(av[:], vxy[:], 0.0, None, op0=OP.is_ge)
            STT(ff[:], vxy[:], 1.0, av[:], op0=OP.add, op1=OP.subtract)
            TT(u0t[:], NM, av[:], op=OP.is_equal)
            STT(u1t[:], av[:], 1.0, NM, op0=OP.add, op1=OP.is_equal)
            TT(du[:], u1t[:], u0t[:], op=OP.subtract)
            TT(du[:], du[:], ff[:], op=OP.mult)
            TT(du[:], du[:], u0t[:], op=OP.add)
            TT(rc[:], du[:, 0:9], du[:, 9:18], op=OP.mult)
            a8v = a8s[s][:]
            for g in range(8):  # pack-0 groups (g<4) first
                TT(a8v[:, 9 * g:9 * (g + 1)], rc[:], MASK[:, 9 * g:9 * (g + 1)],
                   op=OP.mult)

        def emit_rotation_pack(s, p):
            """per-pack rot matmul + DRAM-roundtrip scatter into ltbig."""
            o1 = opool.tile([36, 1024], dt.bfloat16, tag="o1", name=f"o1_{s}{p}")
            for h in range(2):
                rot_ps = psmall.tile([36, 512], dt.float32, tag="mlp",
                                     name=f"rps{s}{p}{h}")
                nc.tensor.matmul(rot_ps[:], a8s[s][:, 36 * p:36 * (p + 1)],
                                 bmat8_t[:, 512 * h:512 * (h + 1)],
                                 start=True, stop=True)
                if h == 0:
                    nc.scalar.copy(o1[:, 0:512], rot_ps[:])
                else:
                    nc.vector.tensor_copy(o1[:, 512:1024], rot_ps[:])
            wd = dscr.tile([36, 1024], dt.bfloat16, tag="wd", name=f"wd{s}{p}")
            weng = nc.sync if p == 0 else nc.scalar
            weng.dma_start(wd[:], o1[:])
            wv = wd[:].rearrange("(g ij) (ci co) -> g ij ci co", ij=9, co=32)
            u = 2 * s + p
            for g4 in range(4):
                dst = ltbig[:, 1152 * u:1152 * (u + 1)][
                    32 * g4:32 * (g4 + 1)].rearrange(
                    "q (ij co) -> q ij co", co=128)[:, :, 32 * g4:32 * (g4 + 1)]
                eng = nc.sync if g4 % 2 == 0 else nc.scalar
                eng.dma_start(dst, wv[g4].transpose([1, 0, 2]))

        def emit_gate_mlp(s):
            m_ps = psmall.tile([64, 1], dt.float32, tag="mlp", name=f"mps{s}")
            nc.tensor.matmul(m_ps[:], MLP1[:, 128:192], pooled[:, 2 * s:2 * s + 1],
                             start=True, stop=False)
            nc.tensor.matmul(m_ps[:], MLP1[:, 192:256],
                             pooled[:, 2 * s + 1:2 * s + 2],
                             start=False, stop=True)
            m_sb = cpool.tile([64, 1], dt.float32, name=f"msb{s}")
            TS(m_sb[:], m_ps[:], BIAS64[:, 1:2], 0.0, op0=OP.add, op1=OP.max)
            for p in range(2):
                mod_ps = psmall.tile([128, 1], dt.float32, tag="mlp",
                                     name=f"modps{s}{p}")
                nc.tensor.matmul(mod_ps[:], MLP2[:, 72 + 128 * p:72 + 128 * (p + 1)],
                                 m_sb[:], start=True, stop=True)
                # sigmoid(z) = 0.5*tanh(z/2)+0.5; weights/bias pre-halved
                th = cpool.tile([128, 1], dt.float32, name=f"th{s}{p}")
                nc.scalar.activation(th[:], mod_ps[:], AF.Tanh,
                                     bias=MB2V[:, p:p + 1])
                TS(mod_sb[:, 2 * p + s:2 * p + s + 1], th[:], 0.5, 0.5,
                   op0=OP.mult, op1=OP.add)

        def emit_conv_chunks(u, chunks):
            s, p = divmod(u, 2)
            x3 = x_tiles[u][:].rearrange("c (h w) -> c h w", w=WP)
            mod_col = mod_sb[:, 2 * p + s:2 * p + s + 1]
            for c8 in chunks:
                ps = pconv.tile([128, 512], dt.float32, tag="cps", name=f"cps{u}_{c8}")
                for sft in range(9):
                    ky, kx = divmod(sft, 3)
                    rhs = x3[:, c8 * 8 + ky:c8 * 8 + ky + 8, kx:kx + W]
                    nc.tensor.matmul(
                        ps[:],
                        ltbig[:, 1152 * u + 128 * sft:1152 * u + 128 * (sft + 1)],
                        rhs, start=(sft == 0), stop=(sft == 8))
                ot = outpool.tile([128, 512], dt.float32, tag="ot", name=f"ot{u}_{c8}")
                nc.scalar.mul(ot[:], ps[:], mod_col)
                eng = nc.sync if c8 % 2 == 0 else nc.scalar
                eng.dma_start(
                    y_flat[s * C + 128 * p:s * C + 128 * (p + 1),
                           512 * c8:512 * (c8 + 1)],
                    ot[:])

        # ---- sample 0 prep ----
        c0, s0 = emit_angle_mlp(0)
        emit_r_build(0, c0, s0)
        emit_rotation_pack(0, 0)
        emit_rotation_pack(0, 1)
        emit_gate_mlp(0)
        emit_pooling1()
        # ---- conv unit 0, sample-1 prep interleaved late enough that its
        # dependencies are always ready ----
        emit_conv_chunks(0, range(0, 5))
        c1, s1 = emit_angle_mlp(1)
        emit_conv_chunks(0, range(5, 7))
        emit_r_build(1, c1, s1)
        emit_rotation_pack(1, 0)
        emit_rotation_pack(1, 1)
        emit_conv_chunks(0, range(7, 8))
        emit_gate_mlp(1)
        emit_conv_chunks(1, range(0, 8))
        emit_conv_chunks(2, range(0, 8))
        emit_conv_chunks(3, range(0, 8))
        ctx.close()

    nc.compile()
    return nc


def _get_nc():
    if "nc" not in _CACHE:
        _CACHE["nc"] = _build_nc()
    return _CACHE["nc"]


def run_on_device(inputs, trace=False, tmpdir=None):
    """Shard, run on 8 cores, gather. Returns (y_full, BassKernelResults)."""
    import ml_dtypes
    from concourse.bass_utils import run_bass_kernel_spmd
    x = np.ascontiguousarray(np.asarray(inputs["x"], np.float32))
    hc = _host_consts(
        inputs["base_kernel"], inputs["ap_w1"], inputs["ap_b1"],
        inputs["ap_w2"], inputs["ap_b2"], inputs["meta_w1"],
        inputs["meta_b1"], inputs["meta_w2"], inputs["meta_b2"])
    nc = _get_nc()
    xpad_full = np.zeros((B, C, HP, WP), ml_dtypes.bfloat16)
    xpad_full[:, :, 1:H + 1, 1:W + 1] = x
    in_maps = []
    for c in range(N_CORES):
        im = {"xs": np.ascontiguousarray(xpad_full[Bc * c:Bc * (c + 1)])}
        im.update(hc)
        in_maps.append(im)
    kw = {}
    if trace:
        kw = dict(trace=True, tmpdir=tmpdir)
    res = run_bass_kernel_spmd(nc, in_maps, core_ids=list(range(N_CORES)), **kw)
    y = np.concatenate([res.results[c]["y"] for c in range(N_CORES)], axis=0)
    return y, res


def kernel(**inputs):
    y, _ = run_on_device(inputs)
    return y
